# revision 5
# baseline (speedup 1.0000x reference)
"""Sharded KNN retrieval (NeighborhoodAggregation) on 8 TRN2 NeuronCores.

Reference computation:
    x   = normalize(features)            # [B, D]
    dis = x @ feat_memory.T              # [B, N]
    dis[b, idx[b]] = global_min          # self-mask
    top5 = top_k(dis, 5).indices
    mean_logits  = mean(pred_memory[top5], axis=1)
    pseudo_labels = argmax(mean_logits, 1)

Device strategy (FAISS-style sharded search, per sharding hint):
  - feat_memory is sharded row-wise across the 8 cores (12500 rows each).
  - Row normalization of `features` is skipped: dis rows are scaled by a
    positive per-row constant, which leaves per-row rankings unchanged, and
    only rankings feed the output.
  - Each core computes dis_local = features @ shard.T with bf16 inputs and
    fp32 PSUM accumulation, and reduces each 2500-wide stripe to its top-8
    (values + indices) with the DVE max/max_index instructions.
  - Host merges the 8*40 candidates per row, drops the self-index (reference
    sets it to the global min, which can never reach top-5 of 100k), rescores
    the best 40 in fp32 for rank robustness, gathers pred_memory and reduces.
"""

import sys

for _p in (
    "/root/.axon_site",
    "/root/.axon_site/_ro/trn_rl_repo",
    "/root/.axon_site/_ro/pypackages",
    "/opt/trn_rl_repo",
    "/opt/pypackages",
):
    if _p not in sys.path:
        sys.path.append(_p)

import numpy as np
import ml_dtypes

# concourse's trace path does `from antenv.axon_hooks import ...`; some
# images lack that module entirely. Provide a None-hook shim so tracing
# degrades gracefully instead of raising.
try:
    import antenv.axon_hooks  # noqa: F401
except ImportError:
    import types

    try:
        import antenv

        _hooks = types.ModuleType("antenv.axon_hooks")
        _hooks._hook = None
        _hooks.set_axon_ntff_profile_hook = lambda h: setattr(_hooks, "_hook", h)
        _hooks.get_axon_ntff_profile_hook = lambda: _hooks._hook
        sys.modules["antenv.axon_hooks"] = _hooks
        antenv.axon_hooks = _hooks
    except ImportError:
        pass

import concourse.bacc as bacc
import concourse.mybir as mybir
from concourse.tile import TileContext
from concourse.bass_utils import run_bass_kernel_spmd

# Problem sizes (hardcoded per contest contract)
B = 1024
D = 1024
N = 100000
C_CORES = 8
NL = N // C_CORES  # 12500 rows of feat_memory per core
K = 5

P = 128
CHUNK = 500  # matmul moving free dim (one PSUM bank holds 512 fp32)
STAGE_CHUNKS = 5  # chunks per top-8 stripe
STAGE = CHUNK * STAGE_CHUNKS  # 2500
N_STAGES = NL // STAGE  # 5
CAND = 8 * N_STAGES  # 40 candidates per row per core

BF16 = mybir.dt.bfloat16
F32 = mybir.dt.float32
U32 = mybir.dt.uint32


def build_nc(b=B, d=D, nl=NL, chunk=CHUNK, stage_chunks=STAGE_CHUNKS):
    """Per-core Bass module: dis = x @ w_shard.T, striped top-8."""
    b_tiles = b // P
    k_tiles = d // P
    chunks = nl // chunk
    n_stages = chunks // stage_chunks
    stage_w = chunk * stage_chunks
    cand_w = 8 * n_stages

    nc = bacc.Bacc("TRN2", target_bir_lowering=False, debug=False)
    xT_d = nc.dram_tensor("xT", [d, b], BF16, kind="ExternalInput")
    wt_d = nc.dram_tensor("wt", [d, nl], BF16, kind="ExternalInput")
    val_d = nc.dram_tensor("cand_val", [b, cand_w], F32, kind="ExternalOutput")
    idx_d = nc.dram_tensor("cand_idx", [b, cand_w], U32, kind="ExternalOutput")

    # [d, n] viewed as [p, ko, n] so the contraction dim lands on partitions
    xT_v = xT_d.ap().rearrange("(ko p) b -> p ko b", p=P)
    wt_v = wt_d.ap().rearrange("(ko p) n -> p ko n", p=P)

    with TileContext(nc) as tc:
        with (
            tc.tile_pool(name="const", bufs=1) as const_pool,
            tc.tile_pool(name="wt", bufs=2) as wt_pool,
            tc.tile_pool(name="stage", bufs=3) as stage_pool,
            tc.tile_pool(name="cand", bufs=1) as cand_pool,
            tc.tile_pool(name="psum", bufs=4, space="PSUM") as psum_pool,
        ):
            xT_sb = const_pool.tile([P, k_tiles, b], BF16, name="xT_sb")
            nc.sync.dma_start(xT_sb[:], xT_v)

            cval = [
                cand_pool.tile([P, cand_w], F32, name=f"cval{bt}")
                for bt in range(b_tiles)
            ]
            cidx = [
                cand_pool.tile([P, cand_w], U32, name=f"cidx{bt}")
                for bt in range(b_tiles)
            ]

            for s in range(n_stages):
                wt_sb = wt_pool.tile([P, k_tiles, stage_w], BF16, name="wt_sb")
                nc.sync.dma_start(
                    wt_sb[:], wt_v[:, :, s * stage_w : (s + 1) * stage_w]
                )
                for bt in range(b_tiles):
                    stage_sb = stage_pool.tile([P, stage_w], F32, name="stage_sb")
                    for c in range(stage_chunks):
                        ps = psum_pool.tile([P, chunk], F32, name="ps")
                        for k in range(k_tiles):
                            nc.tensor.matmul(
                                ps[:],
                                lhsT=xT_sb[:, k, bt * P : (bt + 1) * P],
                                rhs=wt_sb[:, k, c * chunk : (c + 1) * chunk],
                                start=(k == 0),
                                stop=(k == k_tiles - 1),
                            )
                        nc.scalar.copy(stage_sb[:, c * chunk : (c + 1) * chunk], ps[:])
                    vslice = cval[bt][:, s * 8 : (s + 1) * 8]
                    nc.vector.max(out=vslice, in_=stage_sb[:])
                    nc.vector.max_index(
                        out=cidx[bt][:, s * 8 : (s + 1) * 8],
                        in_max=vslice,
                        in_values=stage_sb[:],
                    )

            for bt in range(b_tiles):
                nc.sync.dma_start(val_d.ap()[bt * P : (bt + 1) * P, :], cval[bt][:])
                nc.sync.dma_start(idx_d.ap()[bt * P : (bt + 1) * P, :], cidx[bt][:])

    nc.compile()
    return nc


_NC_CACHE = {}


def _get_nc():
    if "nc" not in _NC_CACHE:
        _NC_CACHE["nc"] = build_nc()
    return _NC_CACHE["nc"]


def _device_candidates(features, feat_memory, **run_kwargs):
    """Run the sharded search; returns (values [B, 8*CAND], global idx [B, 8*CAND])."""
    bf16 = ml_dtypes.bfloat16
    xT = np.ascontiguousarray(features.T).astype(bf16)
    in_maps = []
    for c in range(C_CORES):
        shard = feat_memory[c * NL : (c + 1) * NL]
        wt = np.ascontiguousarray(shard.T).astype(bf16)
        in_maps.append({"xT": xT, "wt": wt})

    nc = _get_nc()
    res = run_bass_kernel_spmd(nc, in_maps, core_ids=list(range(C_CORES)), **run_kwargs)

    vals = np.concatenate([res.results[c]["cand_val"] for c in range(C_CORES)], axis=1)
    lidx = np.concatenate(
        [res.results[c]["cand_idx"].astype(np.int64) for c in range(C_CORES)], axis=1
    )
    # local stage index -> global row index
    base = np.concatenate(
        [
            np.repeat(np.arange(N_STAGES) * STAGE + c * NL, 8)
            for c in range(C_CORES)
        ]
    )
    gidx = lidx + base[None, :]
    _NC_CACHE["last_results"] = res
    return vals, gidx, res


def kernel(features, idx, feat_memory, pred_memory):
    features = np.asarray(features, dtype=np.float32)
    feat_memory = np.asarray(feat_memory, dtype=np.float32)
    pred_memory = np.asarray(pred_memory, dtype=np.float32)
    idx = np.asarray(idx).astype(np.int64)

    vals, gidx, _ = _device_candidates(features, feat_memory)

    # Drop self-index candidates (reference masks them to the global min,
    # which cannot appear in the top-5 of 100k entries).
    vals = np.where(gidx == idx[:, None], -np.inf, vals)

    # Keep the 40 best per row by device (bf16) score, then rescore those
    # exactly in fp32 so close ranks are decided at full precision.
    R = 40
    part = np.argpartition(-vals, R - 1, axis=1)[:, :R]
    cand_i = np.take_along_axis(gidx, part, axis=1)  # [B, R]
    cand_v = np.take_along_axis(vals, part, axis=1)
    cand_vecs = feat_memory[cand_i]  # [B, R, D]
    exact = np.einsum("brd,bd->br", cand_vecs, features, dtype=np.float32)
    exact = np.where(np.isinf(cand_v), -np.inf, exact)

    # top-5, ties broken by smaller global index (jax.lax.top_k convention)
    order = np.lexsort((cand_i, -exact), axis=1)[:, :K]
    top5 = np.take_along_axis(cand_i, order, axis=1)  # [B, K]

    _NC_CACHE["last_top5"] = top5
    mean_logits = pred_memory[top5].mean(axis=1, dtype=np.float32)
    pseudo_labels = np.argmax(mean_logits, axis=1).astype(np.int32)
    return pseudo_labels, mean_logits.astype(np.float32)


# revision 8
# speedup vs baseline: 1.5527x; 1.5527x over previous
"""Sharded KNN retrieval (NeighborhoodAggregation) on 8 TRN2 NeuronCores.

Reference computation:
    x   = normalize(features)            # [B, D]
    dis = x @ feat_memory.T              # [B, N]
    dis[b, idx[b]] = global_min          # self-mask
    top5 = top_k(dis, 5).indices
    mean_logits  = mean(pred_memory[top5], axis=1)
    pseudo_labels = argmax(mean_logits, 1)

Device strategy (FAISS-style sharded search, per sharding hint):
  - feat_memory is sharded row-wise across the 8 cores (12500 rows each).
  - Row normalization of `features` is skipped: dis rows are scaled by a
    positive per-row constant, which leaves per-row rankings unchanged, and
    only rankings feed the output.
  - Each core computes dis_local = features @ shard.T with bf16 inputs and
    fp32 PSUM accumulation, and reduces each 2500-wide stripe to its top-8
    (values + indices) with the DVE max/max_index instructions.
  - Host merges the 8*40 candidates per row, drops the self-index (reference
    sets it to the global min, which can never reach top-5 of 100k), rescores
    the best 40 in fp32 for rank robustness, gathers pred_memory and reduces.
"""

import sys

for _p in (
    "/root/.axon_site",
    "/root/.axon_site/_ro/trn_rl_repo",
    "/root/.axon_site/_ro/pypackages",
    "/opt/trn_rl_repo",
    "/opt/pypackages",
):
    if _p not in sys.path:
        sys.path.append(_p)

import numpy as np
import ml_dtypes

# concourse's trace path does `from antenv.axon_hooks import ...`; some
# images lack that module entirely. Provide a None-hook shim so tracing
# degrades gracefully instead of raising.
try:
    import antenv.axon_hooks  # noqa: F401
except ImportError:
    import types

    try:
        import antenv

        _hooks = types.ModuleType("antenv.axon_hooks")
        _hooks._hook = None
        _hooks.set_axon_ntff_profile_hook = lambda h: setattr(_hooks, "_hook", h)
        _hooks.get_axon_ntff_profile_hook = lambda: _hooks._hook
        sys.modules["antenv.axon_hooks"] = _hooks
        antenv.axon_hooks = _hooks
    except ImportError:
        pass

import concourse.bacc as bacc
import concourse.mybir as mybir
from concourse.tile import TileContext
from concourse.bass_utils import run_bass_kernel_spmd

# Problem sizes (hardcoded per contest contract)
B = 1024
D = 1024
N = 100000
C_CORES = 8
NL = N // C_CORES  # 12500 rows of feat_memory per core
K = 5

P = 128
CHUNK = 500  # matmul moving free dim (one PSUM bank holds 512 fp32)
STAGE_CHUNKS = 5  # chunks per top-8 stripe
STAGE = CHUNK * STAGE_CHUNKS  # 2500
N_STAGES = NL // STAGE  # 5
CAND = 8 * N_STAGES  # 40 candidates per row per core

BF16 = mybir.dt.bfloat16
F32 = mybir.dt.float32
U32 = mybir.dt.uint32
FP8 = mybir.dt.float8e4

IN_DT = FP8  # matmul input dtype (fp8 e4m3 + DoubleRow)
STAGE_DT = BF16  # dis staging dtype for the DVE top-8 scan
W_SCALE = 32.0  # keeps fp8-cast bank rows out of the subnormal range


def _pad16(n):
    return (n + 15) // 16 * 16


def build_nc(b=B, d=D, nl=NL, chunk=CHUNK, stage_chunks=STAGE_CHUNKS,
             in_dt=IN_DT, stage_dt=STAGE_DT):
    """Per-core Bass module: dis = x @ w_shard.T, striped top-8.

    fp8 inputs run the PE in DoubleRow mode: each matmul contracts a pair
    of 128-row k-tiles ([128, 2, M] / [128, 2, N] APs, middle step % 16 == 0).
    """
    b_tiles = b // P
    k_tiles = d // P
    chunks = nl // chunk
    n_stages = chunks // stage_chunks
    stage_w = chunk * stage_chunks
    cand_w = 8 * n_stages
    double_row = in_dt == FP8
    kg = 2 if double_row else 1  # k-tiles per matmul group
    k_groups = k_tiles // kg
    stage_pad = _pad16(stage_w)  # middle-dim step of the wt AP

    nc = bacc.Bacc("TRN2", target_bir_lowering=False, debug=False)
    xT_d = nc.dram_tensor("xT", [d, b], in_dt, kind="ExternalInput")
    wt_d = nc.dram_tensor("wt", [d, nl], in_dt, kind="ExternalInput")
    val_d = nc.dram_tensor("cand_val", [b, cand_w], stage_dt, kind="ExternalOutput")
    idx_d = nc.dram_tensor("cand_idx", [b, cand_w], U32, kind="ExternalOutput")

    # [d, n] viewed as [p, ko, n] so the contraction dim lands on partitions
    xT_v = xT_d.ap().rearrange("(ko p) b -> p ko b", p=P)
    wt_v = wt_d.ap().rearrange("(ko p) n -> p ko n", p=P)

    perf_mode = mybir.MatmulPerfMode.DoubleRow if double_row else None

    with TileContext(nc) as tc:
        with (
            tc.tile_pool(name="const", bufs=1) as const_pool,
            tc.tile_pool(name="wt", bufs=2) as wt_pool,
            tc.tile_pool(name="stage", bufs=3) as stage_pool,
            tc.tile_pool(name="cand", bufs=1) as cand_pool,
            tc.tile_pool(name="psum", bufs=4, space="PSUM") as psum_pool,
        ):
            # per-k-group tiles so the first matmul only waits on small DMAs
            xT_sb = []
            for g in range(k_groups):
                xg = const_pool.tile([P, kg, b], in_dt, name=f"xT_sb{g}")
                nc.sync.dma_start(xg[:], xT_v[:, g * kg : (g + 1) * kg, :])
                xT_sb.append(xg)

            cval = [
                cand_pool.tile([P, cand_w], stage_dt, name=f"cval{bt}")
                for bt in range(b_tiles)
            ]
            cidx = [
                cand_pool.tile([P, cand_w], U32, name=f"cidx{bt}")
                for bt in range(b_tiles)
            ]

            for s in range(n_stages):
                wt_sb = []
                for g in range(k_groups):
                    wg = wt_pool.tile(
                        [P, kg, stage_pad], in_dt, name="wt_sb", tag=f"wt{g}"
                    )
                    nc.sync.dma_start(
                        wg[:, :, :stage_w],
                        wt_v[:, g * kg : (g + 1) * kg, s * stage_w : (s + 1) * stage_w],
                    )
                    wt_sb.append(wg)
                for bt in range(b_tiles):
                    stage_sb = stage_pool.tile([P, stage_w], stage_dt, name="stage_sb")
                    for c in range(stage_chunks):
                        ps = psum_pool.tile([P, chunk], F32, name="ps")
                        for g in range(k_groups):
                            if double_row:
                                lhsT = xT_sb[g][:, :, bt * P : (bt + 1) * P]
                                rhs = wt_sb[g][:, :, c * chunk : (c + 1) * chunk]
                            else:
                                lhsT = xT_sb[g][:, 0, bt * P : (bt + 1) * P]
                                rhs = wt_sb[g][:, 0, c * chunk : (c + 1) * chunk]
                            nc.tensor.matmul(
                                ps[:],
                                lhsT=lhsT,
                                rhs=rhs,
                                start=(g == 0),
                                stop=(g == k_groups - 1),
                                perf_mode=perf_mode,
                            )
                        nc.scalar.copy(stage_sb[:, c * chunk : (c + 1) * chunk], ps[:])
                    vslice = cval[bt][:, s * 8 : (s + 1) * 8]
                    nc.vector.max(out=vslice, in_=stage_sb[:])
                    nc.vector.max_index(
                        out=cidx[bt][:, s * 8 : (s + 1) * 8],
                        in_max=vslice,
                        in_values=stage_sb[:],
                    )

            for bt in range(b_tiles):
                nc.sync.dma_start(val_d.ap()[bt * P : (bt + 1) * P, :], cval[bt][:])
                nc.sync.dma_start(idx_d.ap()[bt * P : (bt + 1) * P, :], cidx[bt][:])

    nc.compile()
    return nc


_NC_CACHE = {}


def _get_nc():
    if "nc" not in _NC_CACHE:
        _NC_CACHE["nc"] = build_nc()
    return _NC_CACHE["nc"]


def _device_candidates(features, feat_memory, **run_kwargs):
    """Run the sharded search; returns (values [B, 8*CAND], global idx [B, 8*CAND])."""
    np_in = mybir.dt.np(IN_DT)
    xT = np.ascontiguousarray(features.T).astype(np_in)
    in_maps = []
    for c in range(C_CORES):
        shard = feat_memory[c * NL : (c + 1) * NL]
        # global positive scale: rank-invariant, avoids fp8 subnormals
        wt = (np.ascontiguousarray(shard.T) * W_SCALE).astype(np_in)
        in_maps.append({"xT": xT, "wt": wt})

    nc = _get_nc()
    res = run_bass_kernel_spmd(nc, in_maps, core_ids=list(range(C_CORES)), **run_kwargs)

    vals = np.concatenate(
        [res.results[c]["cand_val"].astype(np.float32) for c in range(C_CORES)], axis=1
    )
    lidx = np.concatenate(
        [res.results[c]["cand_idx"].astype(np.int64) for c in range(C_CORES)], axis=1
    )
    # local stage index -> global row index
    base = np.concatenate(
        [
            np.repeat(np.arange(N_STAGES) * STAGE + c * NL, 8)
            for c in range(C_CORES)
        ]
    )
    gidx = lidx + base[None, :]
    _NC_CACHE["last_results"] = res
    return vals, gidx, res


def kernel(features, idx, feat_memory, pred_memory):
    features = np.asarray(features, dtype=np.float32)
    feat_memory = np.asarray(feat_memory, dtype=np.float32)
    pred_memory = np.asarray(pred_memory, dtype=np.float32)
    idx = np.asarray(idx).astype(np.int64)

    vals, gidx, _ = _device_candidates(features, feat_memory)

    # Drop self-index candidates (reference masks them to the global min,
    # which cannot appear in the top-5 of 100k entries).
    vals = np.where(gidx == idx[:, None], -np.inf, vals)

    # Keep the 40 best per row by device (bf16) score, then rescore those
    # exactly in fp32 so close ranks are decided at full precision.
    R = 40
    part = np.argpartition(-vals, R - 1, axis=1)[:, :R]
    cand_i = np.take_along_axis(gidx, part, axis=1)  # [B, R]
    cand_v = np.take_along_axis(vals, part, axis=1)
    cand_vecs = feat_memory[cand_i]  # [B, R, D]
    exact = np.einsum("brd,bd->br", cand_vecs, features, dtype=np.float32)
    exact = np.where(np.isinf(cand_v), -np.inf, exact)

    # top-5, ties broken by smaller global index (jax.lax.top_k convention)
    order = np.lexsort((cand_i, -exact), axis=1)[:, :K]
    top5 = np.take_along_axis(cand_i, order, axis=1)  # [B, K]

    _NC_CACHE["last_top5"] = top5
    mean_logits = pred_memory[top5].mean(axis=1, dtype=np.float32)
    pseudo_labels = np.argmax(mean_logits, axis=1).astype(np.int32)
    return pseudo_labels, mean_logits.astype(np.float32)


# revision 12
# speedup vs baseline: 1.9476x; 1.2543x over previous
"""Sharded KNN retrieval (NeighborhoodAggregation) on 8 TRN2 NeuronCores.

Reference computation:
    x   = normalize(features)            # [B, D]
    dis = x @ feat_memory.T              # [B, N]
    dis[b, idx[b]] = global_min          # self-mask
    top5 = top_k(dis, 5).indices
    mean_logits  = mean(pred_memory[top5], axis=1)
    pseudo_labels = argmax(mean_logits, 1)

Device strategy (FAISS-style sharded search, per sharding hint):
  - feat_memory is sharded row-wise across the 8 cores (12500 rows each).
  - Row normalization of `features` is skipped: dis rows are scaled by a
    positive per-row constant, which leaves per-row rankings unchanged, and
    only rankings feed the output.
  - Each core computes dis_local = features @ shard.T with bf16 inputs and
    fp32 PSUM accumulation, and reduces each 2500-wide stripe to its top-8
    (values + indices) with the DVE max/max_index instructions.
  - Host merges the 8*40 candidates per row, drops the self-index (reference
    sets it to the global min, which can never reach top-5 of 100k), rescores
    the best 40 in fp32 for rank robustness, gathers pred_memory and reduces.
"""

import sys

for _p in (
    "/root/.axon_site",
    "/root/.axon_site/_ro/trn_rl_repo",
    "/root/.axon_site/_ro/pypackages",
    "/opt/trn_rl_repo",
    "/opt/pypackages",
):
    if _p not in sys.path:
        sys.path.append(_p)

import numpy as np
import ml_dtypes

# concourse's trace path does `from antenv.axon_hooks import ...`; some
# images lack that module entirely. Provide a None-hook shim so tracing
# degrades gracefully instead of raising.
try:
    import antenv.axon_hooks  # noqa: F401
except ImportError:
    import types

    try:
        import antenv

        _hooks = types.ModuleType("antenv.axon_hooks")
        _hooks._hook = None
        _hooks.set_axon_ntff_profile_hook = lambda h: setattr(_hooks, "_hook", h)
        _hooks.get_axon_ntff_profile_hook = lambda: _hooks._hook
        sys.modules["antenv.axon_hooks"] = _hooks
        antenv.axon_hooks = _hooks
    except ImportError:
        pass

import concourse.bacc as bacc
import concourse.mybir as mybir
from concourse.tile import TileContext
from concourse.bass_utils import run_bass_kernel_spmd

# Problem sizes (hardcoded per contest contract)
B = 1024
D = 1024
N = 100000
C_CORES = 8
NL = N // C_CORES  # 12500 rows of feat_memory per core
K = 5

P = 128
CHUNK = 500  # matmul moving free dim (one PSUM bank holds 512 fp32)
STAGE_CHUNKS = 5  # chunks per top-8 stripe
STAGE = CHUNK * STAGE_CHUNKS  # 2500
N_STAGES = NL // STAGE  # 5
CAND = 8 * N_STAGES  # 40 candidates per row per core

BF16 = mybir.dt.bfloat16
F32 = mybir.dt.float32
U32 = mybir.dt.uint32
FP8 = mybir.dt.float8e4

IN_DT = FP8  # matmul input dtype (fp8 e4m3 + DoubleRow)
STAGE_DT = BF16  # dis staging dtype for the DVE top-8 scan
W_SCALE = 32.0  # keeps fp8-cast bank rows out of the subnormal range


def _pad16(n):
    return (n + 15) // 16 * 16


def build_nc(b=B, d=D, nl=NL, chunk=CHUNK, stage_chunks=STAGE_CHUNKS,
             in_dt=IN_DT, stage_dt=STAGE_DT):
    """Per-core Bass module: dis = x @ w_shard.T, striped top-8.

    fp8 inputs run the PE in DoubleRow mode: each matmul contracts a pair
    of 128-row k-tiles ([128, 2, M] / [128, 2, N] APs, middle step % 16 == 0).
    """
    b_tiles = b // P
    k_tiles = d // P
    chunks = nl // chunk
    n_stages = chunks // stage_chunks
    stage_w = chunk * stage_chunks
    cand_w = 8 * n_stages
    double_row = in_dt == FP8
    kg = 2 if double_row else 1  # k-tiles per matmul group
    k_groups = k_tiles // kg
    stage_pad = _pad16(stage_w)  # middle-dim step of the wt AP

    nc = bacc.Bacc("TRN2", target_bir_lowering=False, debug=False)
    xT_d = nc.dram_tensor("xT", [d, b], in_dt, kind="ExternalInput")
    wt_d = nc.dram_tensor("wt", [d, nl], in_dt, kind="ExternalInput")
    # each candidate is one f32 word: [bf16(exp(dis)) bits | 16-bit local idx]
    val_d = nc.dram_tensor("cand_val", [b, cand_w], F32, kind="ExternalOutput")

    # [d, n] viewed as [p, ko, n] so the contraction dim lands on partitions
    xT_v = xT_d.ap().rearrange("(ko p) b -> p ko b", p=P)
    wt_v = wt_d.ap().rearrange("(ko p) n -> p ko n", p=P)

    perf_mode = mybir.MatmulPerfMode.DoubleRow if double_row else None
    N_COMB = 3

    with TileContext(nc) as tc:
        with (
            tc.tile_pool(name="const", bufs=1) as const_pool,
            tc.tile_pool(name="wt", bufs=2) as wt_pool,
            tc.tile_pool(name="cand", bufs=1) as cand_pool,
            tc.tile_pool(name="psum", bufs=1, space="PSUM") as psum_pool,
        ):
            # Combined-word stripes: lane0 (low u16) = iota prefilled once,
            # lane1 (high u16) = bf16 bits of exp(dis) written per stripe.
            # exp > 0 makes every word a positive f32, so one MAX8 returns
            # the top-8 (score, index) pairs — no FIND_INDEX8 pass needed.
            comb = []
            for i in range(N_COMB):
                ct = cand_pool.tile([P, stage_w], U32, name=f"comb{i}")
                lanes = ct.bitcast(mybir.dt.uint16).rearrange(
                    "p (n two) -> p n two", two=2
                )
                nc.gpsimd.iota(
                    lanes[:, :, 0], pattern=[[1, stage_w]], base=0,
                    channel_multiplier=0,
                )
                comb.append(ct)

            # per-k-group tiles so the first matmul only waits on small DMAs
            xT_sb = []
            for g in range(k_groups):
                xg = const_pool.tile([P, kg, b], in_dt, name=f"xT_sb{g}")
                nc.sync.dma_start(xg[:], xT_v[:, g * kg : (g + 1) * kg, :])
                xT_sb.append(xg)

            cval = [
                cand_pool.tile([P, cand_w], F32, name=f"cval{bt}")
                for bt in range(b_tiles)
            ]

            unit = 0
            for s in range(n_stages):
                wt_sb = []
                for g in range(k_groups):
                    wg = wt_pool.tile(
                        [P, kg, stage_pad], in_dt, name="wt_sb", tag=f"wt{g}"
                    )
                    nc.sync.dma_start(
                        wg[:, :, :stage_w],
                        wt_v[:, g * kg : (g + 1) * kg, s * stage_w : (s + 1) * stage_w],
                    )
                    wt_sb.append(wg)
                for bt in range(b_tiles):
                    # k-group outer, chunk inner: the stationary operand
                    # repeats across chunks, keeping LDWEIGHTS reusable
                    pss = [
                        psum_pool.tile(
                            [P, chunk], F32, name="ps", tag=f"ps{c}",
                            bufs=2 if c < 8 - stage_chunks else 1,
                        )
                        for c in range(stage_chunks)
                    ]
                    for g in range(k_groups):
                        for c in range(stage_chunks):
                            if double_row:
                                lhsT = xT_sb[g][:, :, bt * P : (bt + 1) * P]
                                rhs = wt_sb[g][:, :, c * chunk : (c + 1) * chunk]
                            else:
                                lhsT = xT_sb[g][:, 0, bt * P : (bt + 1) * P]
                                rhs = wt_sb[g][:, 0, c * chunk : (c + 1) * chunk]
                            nc.tensor.matmul(
                                pss[c][:],
                                lhsT=lhsT,
                                rhs=rhs,
                                start=(g == 0),
                                stop=(g == k_groups - 1),
                                perf_mode=perf_mode,
                            )
                    ct = comb[unit % N_COMB]
                    ct_bf = ct.bitcast(mybir.dt.bfloat16).rearrange(
                        "p (n two) -> p n two", two=2
                    )
                    for c in range(stage_chunks):
                        nc.scalar.activation(
                            ct_bf[:, c * chunk : (c + 1) * chunk, 1],
                            pss[c][:],
                            mybir.ActivationFunctionType.Exp,
                            scale=1.0 / W_SCALE,
                        )
                    nc.vector.max(
                        out=cval[bt][:, s * 8 : (s + 1) * 8],
                        in_=ct.bitcast(F32),
                    )
                    unit += 1

            for bt in range(b_tiles):
                nc.sync.dma_start(val_d.ap()[bt * P : (bt + 1) * P, :], cval[bt][:])

    nc.compile()
    return nc


_NC_CACHE = {}


def _get_nc():
    if "nc" not in _NC_CACHE:
        _NC_CACHE["nc"] = build_nc()
    return _NC_CACHE["nc"]


def _device_candidates(features, feat_memory, **run_kwargs):
    """Run the sharded search; returns (values [B, 8*CAND], global idx [B, 8*CAND])."""
    np_in = mybir.dt.np(IN_DT)
    xT = np.ascontiguousarray(features.T).astype(np_in)
    in_maps = []
    for c in range(C_CORES):
        shard = feat_memory[c * NL : (c + 1) * NL]
        # global positive scale: rank-invariant, avoids fp8 subnormals
        wt = (np.ascontiguousarray(shard.T) * W_SCALE).astype(np_in)
        in_maps.append({"xT": xT, "wt": wt})

    nc = _get_nc()
    res = run_bass_kernel_spmd(nc, in_maps, core_ids=list(range(C_CORES)), **run_kwargs)

    # decode combined words: low u16 = local index, high u16 = bf16 exp-score
    words = np.concatenate(
        [
            np.ascontiguousarray(res.results[c]["cand_val"]).view(np.uint32)
            for c in range(C_CORES)
        ],
        axis=1,
    )
    lidx = (words & 0xFFFF).astype(np.int64)
    vals = (
        (words >> 16).astype(np.uint16).view(ml_dtypes.bfloat16).astype(np.float32)
    )
    # local stage index -> global row index
    base = np.concatenate(
        [
            np.repeat(np.arange(N_STAGES) * STAGE + c * NL, 8)
            for c in range(C_CORES)
        ]
    )
    gidx = lidx + base[None, :]
    _NC_CACHE["last_results"] = res
    return vals, gidx, res


def kernel(features, idx, feat_memory, pred_memory):
    features = np.asarray(features, dtype=np.float32)
    feat_memory = np.asarray(feat_memory, dtype=np.float32)
    pred_memory = np.asarray(pred_memory, dtype=np.float32)
    idx = np.asarray(idx).astype(np.int64)

    vals, gidx, _ = _device_candidates(features, feat_memory)

    # Drop self-index candidates (reference masks them to the global min,
    # which cannot appear in the top-5 of 100k entries).
    vals = np.where(gidx == idx[:, None], -np.inf, vals)

    # Keep the 40 best per row by device (bf16) score, then rescore those
    # exactly in fp32 so close ranks are decided at full precision.
    R = 40
    part = np.argpartition(-vals, R - 1, axis=1)[:, :R]
    cand_i = np.take_along_axis(gidx, part, axis=1)  # [B, R]
    cand_v = np.take_along_axis(vals, part, axis=1)
    cand_vecs = feat_memory[cand_i]  # [B, R, D]
    exact = np.einsum("brd,bd->br", cand_vecs, features, dtype=np.float32)
    exact = np.where(np.isinf(cand_v), -np.inf, exact)

    # top-5, ties broken by smaller global index (jax.lax.top_k convention)
    order = np.lexsort((cand_i, -exact), axis=1)[:, :K]
    top5 = np.take_along_axis(cand_i, order, axis=1)  # [B, K]

    _NC_CACHE["last_top5"] = top5
    mean_logits = pred_memory[top5].mean(axis=1, dtype=np.float32)
    pseudo_labels = np.argmax(mean_logits, axis=1).astype(np.int32)
    return pseudo_labels, mean_logits.astype(np.float32)
